# revision 20
# baseline (speedup 1.0000x reference)
"""Causal self-attention (B=1, T=4096, E=1024, H=16, D=64) on 8 TRN2 NeuronCores.

Sharding: tensor-parallel over heads — each core owns 2 heads (128 of the
1024 hidden dims). Each core computes its slice of the QKV projection, a
flash-style causal attention for its 2 heads, and a partial output
projection (rows of w_out for its head dims). The host sums the 8 partial
outputs (the row-parallel all-reduce) and adds b_out.

Matmul inputs are bf16 (1 cycle/row on the PE), accumulation fp32 in PSUM.
x and weights are converted to bf16 on the host (halves their DMA too).

Every matmul keeps K=128 (full PE array rows): the hardware activity
monitor only grants the 2.4 GHz clock when the array is fully driven —
K=64 streams run at 1.2 GHz forever. The per-head d=64 S^T matmuls are
therefore packed: both heads share one matmul via a zero-padded
block-diagonal q layout (qTz), with kT's natural two-head stacking as the
stationary operand.

Per-core dataflow (feature-major throughout; tq blocks of 512 = 2 qc
sub-blocks of 256; tk blocks of 128):
  kT/vT [128, 4096], qTz [128, 2, 4096]  (phase A, K=e chunks of 128)
  V' [tk, 2, 65] = PE-transpose of vT + ones column
  per (512-wide tq block qb):
    per tk block tb:   S^T packed = kT_tb.T @ qTz   [tk, qc, h, 256] PSUM
                       P = exp(0.125 * S^T)         ACT, PSUM->SBUF bf16
                       (diag blocks: affine_select zeroes tq < tk)
      per head h:      O'_h += V'_h.T @ P[:,qc,h,:] [65, 512] PSUM accum
    row 64 of O'_h = softmax denominators (ones column trick);
    normalize: broadcast denom row, fast reciprocal, columnwise scale
    -> UnT [hd=128, t] (both heads stacked)
  out_partial[t,:] = UnT_tile.T @ w_out_rows  (pipelined one qb behind)
"""

import sys

for _p in ("/opt/trn_rl_repo",):
    if _p not in sys.path:
        sys.path.insert(0, _p)

import ml_dtypes
import numpy as np

import concourse.bass as bass  # noqa: F401
import concourse.mybir as mybir
import concourse.tile as tile
from concourse import bacc
from concourse.bass_utils import run_bass_kernel_spmd
from concourse.masks import make_identity

T, E = 4096, 1024
H, D = 16, 64
NCORES = 8
HPC = H // NCORES          # heads per core = 2
HD = HPC * D               # hidden dims per core = 128
NT = T // 512              # 8 tq blocks of 512
NE = E // 128              # 8 e-chunks of 128
NTB = T // 128             # 32 tk blocks of 128

F32 = mybir.dt.float32
BF16 = mybir.dt.bfloat16
NPBF16 = np.dtype(ml_dtypes.bfloat16)
AF = mybir.ActivationFunctionType


def _build_kernel():
    nc = bacc.Bacc("TRN2", target_bir_lowering=False, debug=False)

    xT = nc.dram_tensor("xT", [E, T], BF16, kind="ExternalInput")
    wq = nc.dram_tensor("wq", [E, HD], BF16, kind="ExternalInput")
    wk = nc.dram_tensor("wk", [E, HD], BF16, kind="ExternalInput")
    wv = nc.dram_tensor("wv", [E, HD], BF16, kind="ExternalInput")
    bqkv = nc.dram_tensor("bqkv", [3, HD, 1], F32, kind="ExternalInput")
    wo = nc.dram_tensor("wo", [HD, E], BF16, kind="ExternalInput")
    out = nc.dram_tensor("out", [T, E], F32, kind="ExternalOutput")

    with tile.TileContext(nc) as tc:
        _body(nc, tc, xT, wq, wk, wv, bqkv, wo, out)
    nc.compile()
    return nc


def _body(nc, tc, xT, wq, wk, wv, bqkv, wo, out):
    from contextlib import ExitStack

    ctx = ExitStack()
    with ctx:
        const = ctx.enter_context(tc.tile_pool(name="const", bufs=1))
        big = ctx.enter_context(tc.tile_pool(name="big", bufs=1))
        xpool = ctx.enter_context(tc.tile_pool(name="xp", bufs=16))
        ppool = ctx.enter_context(tc.tile_pool(name="pp", bufs=4))
        opool = ctx.enter_context(tc.tile_pool(name="op", bufs=3))
        small = ctx.enter_context(tc.tile_pool(name="sm", bufs=4))
        ps_mm = ctx.enter_context(tc.tile_pool(name="ps_mm", bufs=2, space="PSUM"))
        ps_o = ctx.enter_context(tc.tile_pool(name="ps_o", bufs=4, space="PSUM"))

        # ---- constants / weights ----
        identb = const.tile([128, 128], BF16)
        make_identity(nc, identb[:])

        wq_sb = const.tile([128, NE, HD], BF16)
        wk_sb = const.tile([128, NE, HD], BF16)
        wv_sb = const.tile([128, NE, HD], BF16)
        for w_dram, w_sb in ((wq, wq_sb), (wk, wk_sb), (wv, wv_sb)):
            nc.sync.dma_start(
                w_sb[:], w_dram[:].rearrange("(a p) c -> p a c", p=128)
            )
        wo_sb = const.tile([128, E], BF16)
        nc.sync.dma_start(wo_sb[:], wo[:])

        bq_sb = const.tile([128, 1], F32)
        bk_sb = const.tile([128, 1], F32)
        bv_sb = const.tile([128, 1], F32)
        nc.sync.dma_start(bq_sb[:], bqkv[0])
        nc.sync.dma_start(bk_sb[:], bqkv[1])
        nc.sync.dma_start(bv_sb[:], bqkv[2])

        # q in zero-padded block-diagonal layout: qTz[0:64, 0, t] = head-0
        # q^T, qTz[64:128, 1, t] = head-1 q^T, zeros elsewhere — so S^T for
        # both heads is one K=128 matmul against kT's natural stacking.
        qTz = big.tile([128, 2, T], BF16)
        kT = big.tile([128, T], BF16)
        vT = big.tile([128, T], BF16)
        # V row-major per (tk block, head) plus a ones column.
        V2 = big.tile([128, NTB, HPC, D + 1], BF16)
        # normalized attention outputs, transposed: rows h*64+d, cols t
        UnT = big.tile([128, T], BF16)

        nc.gpsimd.memset(qTz[0:D, 1, :], 0.0)
        nc.gpsimd.memset(qTz[D:128, 0, :], 0.0)
        nc.gpsimd.memset(V2[:, :, :, D], 1.0)

        # ---- merged pipeline: per step, one QKV^T chunk (phase A) feeds
        # the same-index attention q block (phase B) — the dense projection
        # matmuls fill the PE slack while ACT (the exp) paces the tk loop;
        # out-proj lags two blocks behind ----
        def emit_S(qb, tb):
            psS = ps_mm.tile([128, 1024], F32, tag="mm")
            psSv = psS[:].rearrange("p (qc h f) -> p qc h f", qc=2, h=HPC)
            for qc in range(2):
                f0 = max(0, tb * 128 - qb * 512 - qc * 256)
                if f0 >= 256:
                    continue
                t0 = qb * 512 + qc * 256 + f0
                t1 = qb * 512 + (qc + 1) * 256
                nc.tensor.matmul(
                    psSv[:, qc, :, f0:256],
                    kT[:, tb * 128:(tb + 1) * 128],
                    qTz[:, :, t0:t1],
                    start=True, stop=True,
                )
            return psS, psSv

        for step in range(NT):
            # -- phase A chunk: QKV^T for t columns [step*512, step*512+512)
            tcc = step
            ts512 = slice(tcc * 512, (tcc + 1) * 512)
            xs = []
            for ec in range(NE):
                xsb = xpool.tile([128, 512], BF16, tag="xsb")
                nc.sync.dma_start(xsb[:], xT[ec * 128:(ec + 1) * 128, ts512])
                xs.append(xsb)
            for m, (w_sb, b_sb) in enumerate(
                ((wq_sb, bq_sb), (wk_sb, bk_sb), (wv_sb, bv_sb))
            ):
                ps = ps_mm.tile([128, 1024], F32, tag="mm")
                for ec in range(NE):
                    nc.tensor.matmul(
                        ps[:, 0:512], w_sb[:, ec, :], xs[ec][:],
                        start=(ec == 0), stop=(ec == NE - 1),
                    )
                if m == 0:
                    nc.vector.tensor_scalar_add(
                        qTz[0:D, 0, ts512], ps[0:D, 0:512], b_sb[0:D]
                    )
                    nc.vector.tensor_scalar_add(
                        qTz[D:128, 1, ts512], ps[D:128, 0:512], b_sb[D:128]
                    )
                else:
                    dst = kT if m == 1 else vT
                    nc.vector.tensor_scalar_add(
                        dst[:, ts512], ps[:, 0:512], b_sb[:]
                    )
            # V' transposes for this chunk (K=128 full-width: both heads in
            # one transpose per tk block; keeps the PE array fully driven)
            for tb in range(4 * tcc, 4 * (tcc + 1)):
                pst = ps_o.tile([128, 128], BF16, tag="o")
                nc.tensor.transpose(
                    pst[:], vT[:, tb * 128:(tb + 1) * 128], identb[:]
                )
                nc.vector.tensor_copy(
                    V2[:, tb, :, 0:D],
                    pst[:].rearrange("p (h d) -> p h d", h=HPC),
                )

            # -- phase B block: attention for tq in [qb*512, qb*512+512)
            qb = step
            nblk = 4 * (qb + 1)
            pending = list(range((qb - 2) * 4, (qb - 1) * 4)) if qb > 1 else []
            emit_at = {}
            for k, tt in enumerate(pending):
                emit_at[max(0, (k + 1) * nblk // 4 - 1)] = tt
            pos = []
            for h in range(HPC):
                po = ps_o.tile([D + 1, 512], F32, tag="o")
                pos.append(po)
            Stiles = {0: emit_S(qb, 0)}
            for tb in range(nblk):
                diag = tb >= 4 * qb
                qc1_only = tb * 128 - qb * 512 >= 256
                psS, psSv = Stiles.pop(tb)
                P = ppool.tile([128, 1024], BF16, tag="P")
                Pv = P[:].rearrange("p (qc h f) -> p qc h f", qc=2, h=HPC)
                if qc1_only:
                    # only the qc1 sub-block is live: exp + mask just that
                    f1 = tb * 128 - qb * 512 - 256
                    nc.scalar.activation(
                        Pv[:, 1, :, f1:256], psSv[:, 1, :, f1:256],
                        AF.Exp, scale=0.125,
                    )
                    nc.gpsimd.affine_select(
                        out=Pv[:, 1, :, f1:256], in_=Pv[:, 1, :, f1:256],
                        compare_op=mybir.AluOpType.is_ge,
                        fill=0.0,
                        base=qb * 512 + 256 + f1 - tb * 128,
                        channel_multiplier=-1,
                        pattern=[[0, HPC], [1, 256 - f1]],
                    )
                else:
                    nc.scalar.activation(P[:], psS[:], AF.Exp, scale=0.125)
                    if diag:
                        # keep where tq >= tk:
                        # (qb*512 + qc*256 + f) - (tb*128 + p) >= 0
                        nc.gpsimd.affine_select(
                            out=Pv, in_=Pv,
                            compare_op=mybir.AluOpType.is_ge,
                            fill=0.0,
                            base=qb * 512 - tb * 128,
                            channel_multiplier=-1,
                            pattern=[[256, 2], [0, HPC], [1, 256]],
                        )
                # emit the next S ahead of this block's O' so the PE can
                # run it while ACT finishes the exp above
                if tb + 1 < nblk:
                    Stiles[tb + 1] = emit_S(qb, tb + 1)
                for h in range(HPC):
                    if qc1_only:
                        f1 = tb * 128 - qb * 512 - 256
                        nc.tensor.matmul(
                            pos[h][:, 256 + f1:512],
                            V2[:, tb, h, :],
                            Pv[:, 1, h, f1:256],
                            start=(tb == 0), stop=(tb == nblk - 1),
                        )
                    else:
                        nc.tensor.matmul(
                            pos[h][:, :],
                            V2[:, tb, h, :],
                            Pv[:, :, h, :],
                            start=(tb == 0), stop=(tb == nblk - 1),
                        )
                if tb in emit_at:
                    _outproj_tile(nc, ps_mm, opool, UnT, wo_sb, out,
                                  emit_at[tb])
            # normalize: U = O'[0:64] * (1 / O'[64]) columnwise; the two
            # heads' chains are interleaved by stage so DVE and GpSimd
            # pipeline them instead of running them back-to-back
            drows, rbs, rbrs = [], [], []
            for h in range(HPC):
                drow = small.tile([1, 512], F32, tag="drow")
                nc.vector.tensor_copy(drow[:], pos[h][D:D + 1, :])
                drows.append(drow)
            for h in range(HPC):
                rb = small.tile([D, 512], F32, tag="rb")
                nc.gpsimd.partition_broadcast(rb[:], drows[h][:], channels=D)
                rbs.append(rb)
            for h in range(HPC):
                rbr = small.tile([D, 512], F32, tag="rbr")
                nc.vector.reciprocal_approx_fast(rbr[:], rbs[h][:])
                rbrs.append(rbr)
            for h in range(HPC):
                nc.vector.tensor_mul(
                    UnT[h * D:(h + 1) * D, qb * 512:(qb + 1) * 512],
                    pos[h][0:D, :], rbrs[h][:],
                )
        for tt in range((NT - 2) * 4, NT * 4):
            _outproj_tile(nc, ps_mm, opool, UnT, wo_sb, out, tt)


def _outproj_tile(nc, ps_mm, opool, UnT, wo_sb, out, tt):
    osb2 = opool.tile([128, E], F32, tag="out")
    for half in range(2):
        psc = ps_mm.tile([128, 1024], F32, tag="mm")
        nc.tensor.matmul(
            psc[:, 0:512],
            UnT[:, tt * 128:(tt + 1) * 128],
            wo_sb[:, half * 512:(half + 1) * 512],
            start=True, stop=True,
        )
        nc.vector.tensor_copy(
            osb2[:, half * 512:(half + 1) * 512], psc[:, 0:512]
        )
    nc.sync.dma_start(out[tt * 128:(tt + 1) * 128, :], osb2[:])


_NC_CACHE = None


def _get_nc():
    global _NC_CACHE
    if _NC_CACHE is None:
        _NC_CACHE = _build_kernel()
    return _NC_CACHE


def _make_in_maps(x, w_qkv, b_qkv, w_out):
    x2 = np.asarray(x, dtype=np.float32).reshape(T, E)
    xT = np.ascontiguousarray(x2.T).astype(NPBF16)
    w_qkv = np.asarray(w_qkv, dtype=np.float32)
    b_qkv = np.asarray(b_qkv, dtype=np.float32)
    w_out = np.asarray(w_out, dtype=np.float32)
    in_maps = []
    for c in range(NCORES):
        s = slice(c * HD, (c + 1) * HD)
        in_maps.append({
            "xT": xT,
            "wq": np.ascontiguousarray(
                w_qkv[:, 0 * E + c * HD:0 * E + (c + 1) * HD]).astype(NPBF16),
            "wk": np.ascontiguousarray(
                w_qkv[:, 1 * E + c * HD:1 * E + (c + 1) * HD]).astype(NPBF16),
            "wv": np.ascontiguousarray(
                w_qkv[:, 2 * E + c * HD:2 * E + (c + 1) * HD]).astype(NPBF16),
            "bqkv": np.ascontiguousarray(
                np.stack([
                    b_qkv[0 * E + c * HD:0 * E + (c + 1) * HD],
                    b_qkv[1 * E + c * HD:1 * E + (c + 1) * HD],
                    b_qkv[2 * E + c * HD:2 * E + (c + 1) * HD],
                ]).reshape(3, HD, 1)
            ),
            "wo": np.ascontiguousarray(w_out[s, :]).astype(NPBF16),
        })
    return in_maps


def run_sharded(x, w_qkv, b_qkv, w_out, b_out, trace=False):
    """Run the SPMD kernel; returns (full_output, BassKernelResults)."""
    nc = _get_nc()
    in_maps = _make_in_maps(x, w_qkv, b_qkv, w_out)
    res = run_bass_kernel_spmd(
        nc, in_maps, core_ids=list(range(NCORES)), trace=trace
    )
    acc = np.zeros((T, E), dtype=np.float32)
    for c in range(NCORES):
        acc += res.results[c]["out"]
    acc += np.asarray(b_out, dtype=np.float32)[None, :]
    return acc.reshape(1, T, E), res


def kernel(x, w_qkv, b_qkv, w_out, b_out):
    out, _ = run_sharded(x, w_qkv, b_qkv, w_out, b_out, trace=False)
    return out


# revision 21
# speedup vs baseline: 1.1354x; 1.1354x over previous
"""Causal self-attention (B=1, T=4096, E=1024, H=16, D=64) on 8 TRN2 NeuronCores.

Sharding: tensor-parallel over heads — each core owns 2 heads (128 of the
1024 hidden dims). Each core computes its slice of the QKV projection, a
flash-style causal attention for its 2 heads, and a partial output
projection (rows of w_out for its head dims). The host sums the 8 partial
outputs (the row-parallel all-reduce) and adds b_out.

Matmul inputs are bf16 (1 cycle/row on the PE), accumulation fp32 in PSUM.
x and weights are converted to bf16 on the host (halves their DMA too).

Every matmul keeps K=128 (full PE array rows): the hardware activity
monitor only grants the 2.4 GHz clock when the array is fully driven —
K=64 streams run at 1.2 GHz forever. The per-head d=64 S^T matmuls are
therefore packed: both heads share one matmul via a zero-padded
block-diagonal q layout (qTz), with kT's natural two-head stacking as the
stationary operand.

Per-core dataflow (feature-major throughout; tq blocks of 512 = 2 qc
sub-blocks of 256; tk blocks of 128):
  kT/vT [128, 4096], qTz [128, 2, 4096]  (phase A, K=e chunks of 128)
  V' [tk, 2, 65] = PE-transpose of vT + ones column
  per (512-wide tq block qb):
    per tk block tb:   S^T packed = kT_tb.T @ qTz   [tk, qc, h, 256] PSUM
                       P = exp(0.125 * S^T)         ACT, PSUM->SBUF bf16
                       (diag blocks: affine_select zeroes tq < tk)
      per head h:      O'_h += V'_h.T @ P[:,qc,h,:] [65, 512] PSUM accum
    row 64 of O'_h = softmax denominators (ones column trick);
    normalize: broadcast denom row, fast reciprocal, columnwise scale
    -> UnT [hd=128, t] (both heads stacked)
  out_partial[t,:] = UnT_tile.T @ w_out_rows  (pipelined one qb behind)
"""

import sys

for _p in ("/opt/trn_rl_repo",):
    if _p not in sys.path:
        sys.path.insert(0, _p)

import ml_dtypes
import numpy as np

import concourse.bass as bass  # noqa: F401
import concourse.mybir as mybir
import concourse.tile as tile
from concourse import bacc
from concourse.bass_utils import run_bass_kernel_spmd
from concourse.masks import make_identity

T, E = 4096, 1024
H, D = 16, 64
NCORES = 8
HPC = H // NCORES          # heads per core = 2
HD = HPC * D               # hidden dims per core = 128
NT = T // 512              # 8 tq blocks of 512
NE = E // 128              # 8 e-chunks of 128
NTB = T // 128             # 32 tk blocks of 128

F32 = mybir.dt.float32
BF16 = mybir.dt.bfloat16
NPBF16 = np.dtype(ml_dtypes.bfloat16)
AF = mybir.ActivationFunctionType


def _build_kernel():
    nc = bacc.Bacc("TRN2", target_bir_lowering=False, debug=False)

    xT = nc.dram_tensor("xT", [E, T], BF16, kind="ExternalInput")
    wq = nc.dram_tensor("wq", [E, HD], BF16, kind="ExternalInput")
    wk = nc.dram_tensor("wk", [E, HD], BF16, kind="ExternalInput")
    wv = nc.dram_tensor("wv", [E, HD], BF16, kind="ExternalInput")
    bqkv = nc.dram_tensor("bqkv", [3, HD, 1], F32, kind="ExternalInput")
    wo = nc.dram_tensor("wo", [HD, E], BF16, kind="ExternalInput")
    out = nc.dram_tensor("out", [T, E], F32, kind="ExternalOutput")

    with tile.TileContext(nc) as tc:
        _body(nc, tc, xT, wq, wk, wv, bqkv, wo, out)
    nc.compile()
    return nc


def _body(nc, tc, xT, wq, wk, wv, bqkv, wo, out):
    from contextlib import ExitStack

    ctx = ExitStack()
    with ctx:
        const = ctx.enter_context(tc.tile_pool(name="const", bufs=1))
        big = ctx.enter_context(tc.tile_pool(name="big", bufs=1))
        xpool = ctx.enter_context(tc.tile_pool(name="xp", bufs=24))
        ppool = ctx.enter_context(tc.tile_pool(name="pp", bufs=4))
        opool = ctx.enter_context(tc.tile_pool(name="op", bufs=3))
        small = ctx.enter_context(tc.tile_pool(name="sm", bufs=4))
        ps_mm = ctx.enter_context(tc.tile_pool(name="ps_mm", bufs=2, space="PSUM"))
        ps_o = ctx.enter_context(tc.tile_pool(name="ps_o", bufs=3, space="PSUM"))
        ps_q = ctx.enter_context(tc.tile_pool(name="ps_q", bufs=1, space="PSUM"))

        # ---- constants / weights ----
        identb = const.tile([128, 128], BF16)
        make_identity(nc, identb[:])

        wq_sb = const.tile([128, NE, HD], BF16)
        wk_sb = const.tile([128, NE, HD], BF16)
        wv_sb = const.tile([128, NE, HD], BF16)
        for w_dram, w_sb in ((wq, wq_sb), (wk, wk_sb), (wv, wv_sb)):
            nc.sync.dma_start(
                w_sb[:], w_dram[:].rearrange("(a p) c -> p a c", p=128)
            )
        wo_sb = const.tile([128, E], BF16)
        nc.sync.dma_start(wo_sb[:], wo[:])

        bq_sb = const.tile([128, 1], F32)
        bk_sb = const.tile([128, 1], F32)
        bv_sb = const.tile([128, 1], F32)
        nc.sync.dma_start(bq_sb[:], bqkv[0])
        nc.sync.dma_start(bk_sb[:], bqkv[1])
        nc.sync.dma_start(bv_sb[:], bqkv[2])

        # q in zero-padded block-diagonal layout: qTz[0:64, 0, t] = head-0
        # q^T, qTz[64:128, 1, t] = head-1 q^T, zeros elsewhere — so S^T for
        # both heads is one K=128 matmul against kT's natural stacking.
        qTz = big.tile([128, 2, T], BF16)
        kT = big.tile([128, T], BF16)
        vT = big.tile([128, T], BF16)
        # V row-major per (tk block, head) plus a ones column.
        V2 = big.tile([128, NTB, HPC, D + 1], BF16)
        # normalized attention outputs, transposed: rows h*64+d, cols t
        UnT = big.tile([128, T], BF16)

        nc.gpsimd.memset(qTz[0:D, 1, :], 0.0)
        nc.gpsimd.memset(qTz[D:128, 0, :], 0.0)
        nc.gpsimd.memset(V2[:, :, :, D], 1.0)

        wparams = ((wq_sb, bq_sb), (wk_sb, bk_sb), (wv_sb, bv_sb))
        xs_map = {}

        def load_x(tcc):
            xs = []
            ts512 = slice(tcc * 512, (tcc + 1) * 512)
            for ec in range(NE):
                xsb = xpool.tile([128, 512], BF16, tag="xsb")
                nc.sync.dma_start(xsb[:], xT[ec * 128:(ec + 1) * 128, ts512])
                xs.append(xsb)
            xs_map[tcc] = xs

        def emit_qkv(tcc, m):
            w_sb, b_sb = wparams[m]
            ts512 = slice(tcc * 512, (tcc + 1) * 512)
            ps = ps_q.tile([128, 512], F32, tag="q")
            for ec in range(NE):
                nc.tensor.matmul(
                    ps[:], w_sb[:, ec, :], xs_map[tcc][ec][:],
                    start=(ec == 0), stop=(ec == NE - 1),
                )
            if m == 0:
                nc.vector.tensor_scalar_add(
                    qTz[0:D, 0, ts512], ps[0:D, :], b_sb[0:D]
                )
                nc.vector.tensor_scalar_add(
                    qTz[D:128, 1, ts512], ps[D:128, :], b_sb[D:128]
                )
            else:
                dst = kT if m == 1 else vT
                nc.vector.tensor_scalar_add(dst[:, ts512], ps[:], b_sb[:])

        def emit_vtrans(tcc, j):
            # V' transpose, K=128 full-width: both heads in one go
            tb = 4 * tcc + j
            pst = ps_q.tile([128, 128], BF16, tag="q")
            nc.tensor.transpose(
                pst[:], vT[:, tb * 128:(tb + 1) * 128], identb[:]
            )
            nc.vector.tensor_copy(
                V2[:, tb, :, 0:D],
                pst[:].rearrange("p (h d) -> p h d", h=HPC),
            )

        def emit_piece(piece):
            kind = piece[0]
            if kind == "qkv":
                emit_qkv(piece[1], piece[2])
            elif kind == "vtrans":
                emit_vtrans(piece[1], piece[2])
            else:
                _outproj_tile(nc, ps_mm, opool, UnT, wo_sb, out, piece[1])

        def emit_S(qb, tb):
            psS = ps_mm.tile([128, 1024], F32, tag="mm")
            psSv = psS[:].rearrange("p (qc h f) -> p qc h f", qc=2, h=HPC)
            for qc in range(2):
                f0 = max(0, tb * 128 - qb * 512 - qc * 256)
                if f0 >= 256:
                    continue
                t0 = qb * 512 + qc * 256 + f0
                t1 = qb * 512 + (qc + 1) * 256
                nc.tensor.matmul(
                    psSv[:, qc, :, f0:256],
                    kT[:, tb * 128:(tb + 1) * 128],
                    qTz[:, :, t0:t1],
                    start=True, stop=True,
                )
            return psS, psSv

        # ---- prologue: x for chunks 0/1, full QKV chunk 0 ----
        load_x(0)
        load_x(1)
        for m in range(3):
            emit_qkv(0, m)
        for j in range(4):
            emit_vtrans(0, j)

        # ---- merged pipeline: per step, the attention q block `step`
        # runs its ACT-paced tk loop while the NEXT chunk's projection
        # matmuls and the lag-2 out-proj are spread through it as PE
        # filler pieces ----
        for step in range(NT):
            if step + 2 < NT:
                load_x(step + 2)
            pieces = []
            if step + 1 < NT:
                pieces += [("qkv", step + 1, m) for m in range(3)]
                pieces += [("vtrans", step + 1, j) for j in range(4)]
            if step >= 2:
                pieces += [("out", tt)
                           for tt in range((step - 2) * 4, (step - 1) * 4)]
            qb = step
            nblk = 4 * (qb + 1)
            emit_at = {}
            for i, piece in enumerate(pieces):
                emit_at.setdefault((i + 1) * nblk // (len(pieces) + 1),
                                   []).append(piece)
            pos = []
            for h in range(HPC):
                po = ps_o.tile([D + 1, 512], F32, tag="o")
                pos.append(po)
            Stiles = {0: emit_S(qb, 0)}
            for tb in range(nblk):
                diag = tb >= 4 * qb
                qc1_only = tb * 128 - qb * 512 >= 256
                psS, psSv = Stiles.pop(tb)
                P = ppool.tile([128, 1024], BF16, tag="P")
                Pv = P[:].rearrange("p (qc h f) -> p qc h f", qc=2, h=HPC)
                if qc1_only:
                    # only the qc1 sub-block is live: exp + mask just that
                    f1 = tb * 128 - qb * 512 - 256
                    nc.scalar.activation(
                        Pv[:, 1, :, f1:256], psSv[:, 1, :, f1:256],
                        AF.Exp, scale=0.125,
                    )
                    nc.gpsimd.affine_select(
                        out=Pv[:, 1, :, f1:256], in_=Pv[:, 1, :, f1:256],
                        compare_op=mybir.AluOpType.is_ge,
                        fill=0.0,
                        base=qb * 512 + 256 + f1 - tb * 128,
                        channel_multiplier=-1,
                        pattern=[[0, HPC], [1, 256 - f1]],
                    )
                else:
                    nc.scalar.activation(P[:], psS[:], AF.Exp, scale=0.125)
                    if diag:
                        # keep where tq >= tk:
                        # (qb*512 + qc*256 + f) - (tb*128 + p) >= 0
                        nc.gpsimd.affine_select(
                            out=Pv, in_=Pv,
                            compare_op=mybir.AluOpType.is_ge,
                            fill=0.0,
                            base=qb * 512 - tb * 128,
                            channel_multiplier=-1,
                            pattern=[[256, 2], [0, HPC], [1, 256]],
                        )
                # emit the next S ahead of this block's O' so the PE can
                # run it while ACT finishes the exp above
                if tb + 1 < nblk:
                    Stiles[tb + 1] = emit_S(qb, tb + 1)
                for h in range(HPC):
                    if qc1_only:
                        f1 = tb * 128 - qb * 512 - 256
                        nc.tensor.matmul(
                            pos[h][:, 256 + f1:512],
                            V2[:, tb, h, :],
                            Pv[:, 1, h, f1:256],
                            start=(tb == 0), stop=(tb == nblk - 1),
                        )
                    else:
                        nc.tensor.matmul(
                            pos[h][:, :],
                            V2[:, tb, h, :],
                            Pv[:, :, h, :],
                            start=(tb == 0), stop=(tb == nblk - 1),
                        )
                for piece in emit_at.get(tb, ()):
                    emit_piece(piece)
            # normalize: U = O'[0:64] * (1 / O'[64]) columnwise; the two
            # heads' chains are interleaved by stage so DVE and GpSimd
            # pipeline them instead of running them back-to-back
            drows, rbs, rbrs = [], [], []
            for h in range(HPC):
                drow = small.tile([1, 512], F32, tag="drow")
                nc.vector.tensor_copy(drow[:], pos[h][D:D + 1, :])
                drows.append(drow)
            for h in range(HPC):
                rb = small.tile([D, 512], F32, tag="rb")
                nc.gpsimd.partition_broadcast(rb[:], drows[h][:], channels=D)
                rbs.append(rb)
            for h in range(HPC):
                rbr = small.tile([D, 512], F32, tag="rbr")
                nc.vector.reciprocal_approx_fast(rbr[:], rbs[h][:])
                rbrs.append(rbr)
            for h in range(HPC):
                nc.vector.tensor_mul(
                    UnT[h * D:(h + 1) * D, qb * 512:(qb + 1) * 512],
                    pos[h][0:D, :], rbrs[h][:],
                )
        for tt in range((NT - 2) * 4, NT * 4):
            _outproj_tile(nc, ps_mm, opool, UnT, wo_sb, out, tt)


def _outproj_tile(nc, ps_mm, opool, UnT, wo_sb, out, tt):
    osb2 = opool.tile([128, E], F32, tag="out")
    for half in range(2):
        psc = ps_mm.tile([128, 1024], F32, tag="mm")
        nc.tensor.matmul(
            psc[:, 0:512],
            UnT[:, tt * 128:(tt + 1) * 128],
            wo_sb[:, half * 512:(half + 1) * 512],
            start=True, stop=True,
        )
        nc.vector.tensor_copy(
            osb2[:, half * 512:(half + 1) * 512], psc[:, 0:512]
        )
    nc.sync.dma_start(out[tt * 128:(tt + 1) * 128, :], osb2[:])


_NC_CACHE = None


def _get_nc():
    global _NC_CACHE
    if _NC_CACHE is None:
        _NC_CACHE = _build_kernel()
    return _NC_CACHE


def _make_in_maps(x, w_qkv, b_qkv, w_out):
    x2 = np.asarray(x, dtype=np.float32).reshape(T, E)
    xT = np.ascontiguousarray(x2.T).astype(NPBF16)
    w_qkv = np.asarray(w_qkv, dtype=np.float32)
    b_qkv = np.asarray(b_qkv, dtype=np.float32)
    w_out = np.asarray(w_out, dtype=np.float32)
    in_maps = []
    for c in range(NCORES):
        s = slice(c * HD, (c + 1) * HD)
        in_maps.append({
            "xT": xT,
            "wq": np.ascontiguousarray(
                w_qkv[:, 0 * E + c * HD:0 * E + (c + 1) * HD]).astype(NPBF16),
            "wk": np.ascontiguousarray(
                w_qkv[:, 1 * E + c * HD:1 * E + (c + 1) * HD]).astype(NPBF16),
            "wv": np.ascontiguousarray(
                w_qkv[:, 2 * E + c * HD:2 * E + (c + 1) * HD]).astype(NPBF16),
            "bqkv": np.ascontiguousarray(
                np.stack([
                    b_qkv[0 * E + c * HD:0 * E + (c + 1) * HD],
                    b_qkv[1 * E + c * HD:1 * E + (c + 1) * HD],
                    b_qkv[2 * E + c * HD:2 * E + (c + 1) * HD],
                ]).reshape(3, HD, 1)
            ),
            "wo": np.ascontiguousarray(w_out[s, :]).astype(NPBF16),
        })
    return in_maps


def run_sharded(x, w_qkv, b_qkv, w_out, b_out, trace=False):
    """Run the SPMD kernel; returns (full_output, BassKernelResults)."""
    nc = _get_nc()
    in_maps = _make_in_maps(x, w_qkv, b_qkv, w_out)
    res = run_bass_kernel_spmd(
        nc, in_maps, core_ids=list(range(NCORES)), trace=trace
    )
    acc = np.zeros((T, E), dtype=np.float32)
    for c in range(NCORES):
        acc += res.results[c]["out"]
    acc += np.asarray(b_out, dtype=np.float32)[None, :]
    return acc.reshape(1, T, E), res


def kernel(x, w_qkv, b_qkv, w_out, b_out):
    out, _ = run_sharded(x, w_qkv, b_qkv, w_out, b_out, trace=False)
    return out


# revision 25
# speedup vs baseline: 1.1360x; 1.0005x over previous
"""Causal self-attention (B=1, T=4096, E=1024, H=16, D=64) on 8 TRN2 NeuronCores.

Sharding: tensor-parallel over heads — each core owns 2 heads (128 of the
1024 hidden dims). Each core computes its slice of the QKV projection, a
flash-style causal attention for its 2 heads, and a partial output
projection (rows of w_out for its head dims). The host sums the 8 partial
outputs (the row-parallel all-reduce) and adds b_out.

Matmul inputs are bf16 (1 cycle/row on the PE), accumulation fp32 in PSUM.
x and weights are converted to bf16 on the host (halves their DMA too).

Every matmul keeps K=128 (full PE array rows): the hardware activity
monitor only grants the 2.4 GHz clock when the array is fully driven —
K=64 streams run at 1.2 GHz forever. The per-head d=64 S^T matmuls are
therefore packed: both heads share one matmul via a zero-padded
block-diagonal q layout (qTz), with kT's natural two-head stacking as the
stationary operand.

Per-core dataflow (feature-major throughout; tq blocks of 512 = 2 qc
sub-blocks of 256; tk blocks of 128):
  kT/vT [128, 4096], qTz [128, 2, 4096]  (phase A, K=e chunks of 128)
  V' [tk, 2, 65] = PE-transpose of vT + ones column
  per (512-wide tq block qb):
    per tk block tb:   S^T packed = kT_tb.T @ qTz   [tk, qc, h, 256] PSUM
                       P = exp(0.125 * S^T)         ACT, PSUM->SBUF bf16
                       (diag blocks: affine_select zeroes tq < tk)
      per head h:      O'_h += V'_h.T @ P[:,qc,h,:] [65, 512] PSUM accum
    row 64 of O'_h = softmax denominators (ones column trick);
    normalize: broadcast denom row, fast reciprocal, columnwise scale
    -> UnT [hd=128, t] (both heads stacked)
  out_partial[t,:] = UnT_tile.T @ w_out_rows  (pipelined one qb behind)
"""

import sys

for _p in ("/opt/trn_rl_repo",):
    if _p not in sys.path:
        sys.path.insert(0, _p)

import ml_dtypes
import numpy as np

import concourse.bass as bass  # noqa: F401
import concourse.mybir as mybir
import concourse.tile as tile
from concourse import bacc
from concourse.bass_utils import run_bass_kernel_spmd
from concourse.masks import make_identity

T, E = 4096, 1024
H, D = 16, 64
NCORES = 8
HPC = H // NCORES          # heads per core = 2
HD = HPC * D               # hidden dims per core = 128
NT = T // 512              # 8 tq blocks of 512
NE = E // 128              # 8 e-chunks of 128
NTB = T // 128             # 32 tk blocks of 128

F32 = mybir.dt.float32
BF16 = mybir.dt.bfloat16
NPBF16 = np.dtype(ml_dtypes.bfloat16)
AF = mybir.ActivationFunctionType


def _build_kernel():
    nc = bacc.Bacc("TRN2", target_bir_lowering=False, debug=False)

    xT = nc.dram_tensor("xT", [E, T], BF16, kind="ExternalInput")
    wq = nc.dram_tensor("wq", [E, HD], BF16, kind="ExternalInput")
    wk = nc.dram_tensor("wk", [E, HD], BF16, kind="ExternalInput")
    wv = nc.dram_tensor("wv", [E, HD], BF16, kind="ExternalInput")
    bqkv = nc.dram_tensor("bqkv", [3, HD, 1], F32, kind="ExternalInput")
    wo = nc.dram_tensor("wo", [HD, E], BF16, kind="ExternalInput")
    out = nc.dram_tensor("out", [T, E], F32, kind="ExternalOutput")

    with tile.TileContext(nc) as tc:
        _body(nc, tc, xT, wq, wk, wv, bqkv, wo, out)
    nc.compile()
    return nc


def _body(nc, tc, xT, wq, wk, wv, bqkv, wo, out):
    from contextlib import ExitStack

    ctx = ExitStack()
    with ctx:
        const = ctx.enter_context(tc.tile_pool(name="const", bufs=1))
        big = ctx.enter_context(tc.tile_pool(name="big", bufs=1))
        xpool = ctx.enter_context(tc.tile_pool(name="xp", bufs=24))
        ppool = ctx.enter_context(tc.tile_pool(name="pp", bufs=4))
        opool = ctx.enter_context(tc.tile_pool(name="op", bufs=3))
        small = ctx.enter_context(tc.tile_pool(name="sm", bufs=4))
        ps_mm = ctx.enter_context(tc.tile_pool(name="ps_mm", bufs=2, space="PSUM"))
        ps_o = ctx.enter_context(tc.tile_pool(name="ps_o", bufs=3, space="PSUM"))
        ps_q = ctx.enter_context(tc.tile_pool(name="ps_q", bufs=1, space="PSUM"))

        # ---- constants / weights ----
        identb = const.tile([128, 128], BF16)
        make_identity(nc, identb[:])

        wq_sb = const.tile([128, NE, HD], BF16)
        wk_sb = const.tile([128, NE, HD], BF16)
        wv_sb = const.tile([128, NE, HD], BF16)
        for w_dram, w_sb in ((wq, wq_sb), (wk, wk_sb), (wv, wv_sb)):
            nc.sync.dma_start(
                w_sb[:], w_dram[:].rearrange("(a p) c -> p a c", p=128)
            )
        wo_sb = const.tile([128, E], BF16)
        nc.sync.dma_start(wo_sb[:], wo[:])

        bq_sb = const.tile([128, 1], F32)
        bk_sb = const.tile([128, 1], F32)
        bv_sb = const.tile([128, 1], F32)
        nc.sync.dma_start(bq_sb[:], bqkv[0])
        nc.sync.dma_start(bk_sb[:], bqkv[1])
        nc.sync.dma_start(bv_sb[:], bqkv[2])

        # q in zero-padded block-diagonal layout: qTz[0:64, 0, t] = head-0
        # q^T, qTz[64:128, 1, t] = head-1 q^T, zeros elsewhere — so S^T for
        # both heads is one K=128 matmul against kT's natural stacking.
        qTz = big.tile([128, 2, T], BF16)
        kT = big.tile([128, T], BF16)
        vT = big.tile([128, T], BF16)
        # V row-major per (tk block, head) plus a ones column.
        V2 = big.tile([128, NTB, HPC, D + 1], BF16)
        # normalized attention outputs, transposed: rows h*64+d, cols t
        UnT = big.tile([128, T], BF16)

        nc.gpsimd.memset(qTz[0:D, 1, :], 0.0)
        nc.gpsimd.memset(qTz[D:128, 0, :], 0.0)
        nc.gpsimd.memset(V2[:, :, :, D], 1.0)

        wparams = ((wq_sb, bq_sb), (wk_sb, bk_sb), (wv_sb, bv_sb))
        xs_map = {}

        def load_x(tcc):
            xs = []
            ts512 = slice(tcc * 512, (tcc + 1) * 512)
            for ec in range(NE):
                xsb = xpool.tile([128, 512], BF16, tag="xsb")
                nc.sync.dma_start(xsb[:], xT[ec * 128:(ec + 1) * 128, ts512])
                xs.append(xsb)
            xs_map[tcc] = xs

        def emit_qkv(tcc, m):
            w_sb, b_sb = wparams[m]
            ts512 = slice(tcc * 512, (tcc + 1) * 512)
            ps = ps_q.tile([128, 512], F32, tag="q")
            for ec in range(NE):
                nc.tensor.matmul(
                    ps[:], w_sb[:, ec, :], xs_map[tcc][ec][:],
                    start=(ec == 0), stop=(ec == NE - 1),
                )
            if m == 0:
                nc.vector.tensor_scalar_add(
                    qTz[0:D, 0, ts512], ps[0:D, :], b_sb[0:D]
                )
                nc.vector.tensor_scalar_add(
                    qTz[D:128, 1, ts512], ps[D:128, :], b_sb[D:128]
                )
            else:
                dst = kT if m == 1 else vT
                nc.vector.tensor_scalar_add(dst[:, ts512], ps[:], b_sb[:])

        def emit_vtrans(tcc, j):
            # V' transpose, K=128 full-width: both heads in one go
            tb = 4 * tcc + j
            pst = ps_q.tile([128, 128], BF16, tag="q")
            nc.tensor.transpose(
                pst[:], vT[:, tb * 128:(tb + 1) * 128], identb[:]
            )
            nc.vector.tensor_copy(
                V2[:, tb, :, 0:D],
                pst[:].rearrange("p (h d) -> p h d", h=HPC),
            )

        def emit_piece(piece):
            kind = piece[0]
            if kind == "qkv":
                emit_qkv(piece[1], piece[2])
            elif kind == "vtrans":
                emit_vtrans(piece[1], piece[2])
            else:
                _outproj_tile(nc, ps_mm, opool, UnT, wo_sb, out, piece[1])

        def emit_S(qb, tb):
            psS = ps_mm.tile([128, 1024], F32, tag="mm")
            psSv = psS[:].rearrange("p (qc h f) -> p qc h f", qc=2, h=HPC)
            for qc in range(2):
                f0 = max(0, tb * 128 - qb * 512 - qc * 256)
                if f0 >= 256:
                    continue
                t0 = qb * 512 + qc * 256 + f0
                t1 = qb * 512 + (qc + 1) * 256
                nc.tensor.matmul(
                    psSv[:, qc, :, f0:256],
                    kT[:, tb * 128:(tb + 1) * 128],
                    qTz[:, :, t0:t1],
                    start=True, stop=True,
                )
            return psS, psSv

        # ---- prologue: x for chunks 0/1, full QKV chunk 0 ----
        load_x(0)
        load_x(1)
        for m in range(3):
            emit_qkv(0, m)
        for j in range(4):
            emit_vtrans(0, j)

        # ---- merged pipeline: per step, the attention q block `step`
        # runs its ACT-paced tk loop while the NEXT chunk's projection
        # matmuls and the lag-2 out-proj are spread through it as PE
        # filler pieces ----
        for step in range(NT):
            if step + 2 < NT:
                load_x(step + 2)
            pieces = []
            if step + 1 < NT:
                pieces += [("qkv", step + 1, m) for m in range(3)]
                pieces += [("vtrans", step + 1, j) for j in range(4)]
            if step >= 2:
                pieces += [("out", tt)
                           for tt in range((step - 2) * 4, (step - 1) * 4)]
            if step == NT - 1:
                # the second-to-last block's out-proj rides late in the
                # final tk loop (its normalize is settled well before)
                pieces += [("out", tt)
                           for tt in range((NT - 2) * 4, (NT - 1) * 4)]
            qb = step
            nblk = 4 * (qb + 1)
            emit_at = {}
            for i, piece in enumerate(pieces):
                emit_at.setdefault((i + 1) * nblk // (len(pieces) + 1),
                                   []).append(piece)
            pos = []
            for h in range(HPC):
                po = ps_o.tile([D + 1, 512], F32, tag="o")
                pos.append(po)
            Stiles = {0: emit_S(qb, 0)}
            for tb in range(nblk):
                diag = tb >= 4 * qb
                qc1_only = tb * 128 - qb * 512 >= 256
                psS, psSv = Stiles.pop(tb)
                P = ppool.tile([128, 1024], BF16, tag="P")
                Pv = P[:].rearrange("p (qc h f) -> p qc h f", qc=2, h=HPC)
                if qc1_only:
                    # only the qc1 sub-block is live: exp + mask just that
                    f1 = tb * 128 - qb * 512 - 256
                    nc.scalar.activation(
                        Pv[:, 1, :, f1:256], psSv[:, 1, :, f1:256],
                        AF.Exp, scale=0.125,
                    )
                    nc.gpsimd.affine_select(
                        out=Pv[:, 1, :, f1:256], in_=Pv[:, 1, :, f1:256],
                        compare_op=mybir.AluOpType.is_ge,
                        fill=0.0,
                        base=qb * 512 + 256 + f1 - tb * 128,
                        channel_multiplier=-1,
                        pattern=[[0, HPC], [1, 256 - f1]],
                    )
                else:
                    nc.scalar.activation(P[:], psS[:], AF.Exp, scale=0.125)
                    if diag:
                        # keep where tq >= tk:
                        # (qb*512 + qc*256 + f) - (tb*128 + p) >= 0
                        nc.gpsimd.affine_select(
                            out=Pv, in_=Pv,
                            compare_op=mybir.AluOpType.is_ge,
                            fill=0.0,
                            base=qb * 512 - tb * 128,
                            channel_multiplier=-1,
                            pattern=[[256, 2], [0, HPC], [1, 256]],
                        )
                # emit the next S plus any filler pieces ahead of this
                # block's O' — the O' must wait for the exp above to
                # finish, and these keep the PE busy through that wait
                if tb + 1 < nblk:
                    Stiles[tb + 1] = emit_S(qb, tb + 1)
                for piece in emit_at.get(tb, ()):
                    emit_piece(piece)
                for h in range(HPC):
                    if qc1_only:
                        f1 = tb * 128 - qb * 512 - 256
                        nc.tensor.matmul(
                            pos[h][:, 256 + f1:512],
                            V2[:, tb, h, :],
                            Pv[:, 1, h, f1:256],
                            start=(tb == 0), stop=(tb == nblk - 1),
                        )
                    else:
                        nc.tensor.matmul(
                            pos[h][:, :],
                            V2[:, tb, h, :],
                            Pv[:, :, h, :],
                            start=(tb == 0), stop=(tb == nblk - 1),
                        )
            # normalize: U = O'[0:64] * (1 / O'[64]) columnwise; the two
            # heads' chains are interleaved by stage so DVE and GpSimd
            # pipeline them instead of running them back-to-back
            drows, rbs, rbrs = [], [], []
            for h in range(HPC):
                drow = small.tile([1, 512], F32, tag="drow")
                nc.vector.tensor_copy(drow[:], pos[h][D:D + 1, :])
                drows.append(drow)
            for h in range(HPC):
                rb = small.tile([D, 512], F32, tag="rb")
                nc.gpsimd.partition_broadcast(rb[:], drows[h][:], channels=D)
                rbs.append(rb)
            for h in range(HPC):
                rbr = small.tile([D, 512], F32, tag="rbr")
                nc.vector.reciprocal_approx_fast(rbr[:], rbs[h][:])
                rbrs.append(rbr)
            for h in range(HPC):
                nc.vector.tensor_mul(
                    UnT[h * D:(h + 1) * D, qb * 512:(qb + 1) * 512],
                    pos[h][0:D, :], rbrs[h][:],
                )
        for tt in range((NT - 1) * 4, NT * 4):
            _outproj_tile(nc, ps_mm, opool, UnT, wo_sb, out, tt)


def _outproj_tile(nc, ps_mm, opool, UnT, wo_sb, out, tt):
    osb2 = opool.tile([128, E], F32, tag="out")
    for half in range(2):
        psc = ps_mm.tile([128, 1024], F32, tag="mm")
        nc.tensor.matmul(
            psc[:, 0:512],
            UnT[:, tt * 128:(tt + 1) * 128],
            wo_sb[:, half * 512:(half + 1) * 512],
            start=True, stop=True,
        )
        nc.vector.tensor_copy(
            osb2[:, half * 512:(half + 1) * 512], psc[:, 0:512]
        )
    nc.sync.dma_start(out[tt * 128:(tt + 1) * 128, :], osb2[:])


_NC_CACHE = None


def _get_nc():
    global _NC_CACHE
    if _NC_CACHE is None:
        _NC_CACHE = _build_kernel()
    return _NC_CACHE


def _make_in_maps(x, w_qkv, b_qkv, w_out):
    x2 = np.asarray(x, dtype=np.float32).reshape(T, E)
    xT = np.ascontiguousarray(x2.T).astype(NPBF16)
    w_qkv = np.asarray(w_qkv, dtype=np.float32)
    b_qkv = np.asarray(b_qkv, dtype=np.float32)
    w_out = np.asarray(w_out, dtype=np.float32)
    in_maps = []
    for c in range(NCORES):
        s = slice(c * HD, (c + 1) * HD)
        in_maps.append({
            "xT": xT,
            "wq": np.ascontiguousarray(
                w_qkv[:, 0 * E + c * HD:0 * E + (c + 1) * HD]).astype(NPBF16),
            "wk": np.ascontiguousarray(
                w_qkv[:, 1 * E + c * HD:1 * E + (c + 1) * HD]).astype(NPBF16),
            "wv": np.ascontiguousarray(
                w_qkv[:, 2 * E + c * HD:2 * E + (c + 1) * HD]).astype(NPBF16),
            "bqkv": np.ascontiguousarray(
                np.stack([
                    b_qkv[0 * E + c * HD:0 * E + (c + 1) * HD],
                    b_qkv[1 * E + c * HD:1 * E + (c + 1) * HD],
                    b_qkv[2 * E + c * HD:2 * E + (c + 1) * HD],
                ]).reshape(3, HD, 1)
            ),
            "wo": np.ascontiguousarray(w_out[s, :]).astype(NPBF16),
        })
    return in_maps


def run_sharded(x, w_qkv, b_qkv, w_out, b_out, trace=False):
    """Run the SPMD kernel; returns (full_output, BassKernelResults)."""
    nc = _get_nc()
    in_maps = _make_in_maps(x, w_qkv, b_qkv, w_out)
    res = run_bass_kernel_spmd(
        nc, in_maps, core_ids=list(range(NCORES)), trace=trace
    )
    acc = np.zeros((T, E), dtype=np.float32)
    for c in range(NCORES):
        acc += res.results[c]["out"]
    acc += np.asarray(b_out, dtype=np.float32)[None, :]
    return acc.reshape(1, T, E), res


def kernel(x, w_qkv, b_qkv, w_out, b_out):
    out, _ = run_sharded(x, w_qkv, b_qkv, w_out, b_out, trace=False)
    return out


# revision 30
# speedup vs baseline: 1.2086x; 1.0639x over previous
"""Causal self-attention (B=1, T=4096, E=1024, H=16, D=64) on 8 TRN2 NeuronCores.

Sharding: tensor-parallel over heads — each core owns 2 heads (128 of the
1024 hidden dims). Each core computes its slice of the QKV projection, a
flash-style causal attention for its 2 heads, and a partial output
projection (rows of w_out for its head dims). The host sums the 8 partial
outputs (the row-parallel all-reduce) and adds b_out.

Matmul inputs are bf16 (1 cycle/row on the PE), accumulation fp32 in PSUM.
x and weights are converted to bf16 on the host (halves their DMA too).

Every matmul keeps K=128 (full PE array rows): the hardware activity
monitor only grants the 2.4 GHz clock when the array is fully driven —
K=64 streams run at 1.2 GHz forever. The per-head d=64 S^T matmuls are
therefore packed: both heads share one matmul via a zero-padded
block-diagonal q layout (qTz), with kT's natural two-head stacking as the
stationary operand.

Per-core dataflow (feature-major throughout; tq blocks of 512 = 2 qc
sub-blocks of 256; tk blocks of 128):
  kT/vT [128, 4096], qTz [128, 2, 4096]  (phase A, K=e chunks of 128)
  V' [tk, 2, 65] = PE-transpose of vT + ones column
  per (512-wide tq block qb):
    per tk block tb:   S^T packed = kT_tb.T @ qTz   [tk, qc, h, 256] PSUM
                       P = exp(0.125 * S^T)         ACT, PSUM->SBUF bf16
                       (diag blocks: affine_select zeroes tq < tk)
      per head h:      O'_h += V'_h.T @ P[:,qc,h,:] [65, 512] PSUM accum
    row 64 of O'_h = softmax denominators (ones column trick);
    normalize: broadcast denom row, fast reciprocal, columnwise scale
    -> UnT [hd=128, t] (both heads stacked)
  out_partial[t,:] = UnT_tile.T @ w_out_rows  (pipelined one qb behind)
"""

import sys

for _p in ("/opt/trn_rl_repo",):
    if _p not in sys.path:
        sys.path.insert(0, _p)

import ml_dtypes
import numpy as np

import concourse.bass as bass  # noqa: F401
import concourse.mybir as mybir
import concourse.tile as tile
from concourse import bacc
from concourse.bass_utils import run_bass_kernel_spmd
from concourse.masks import make_identity

T, E = 4096, 1024
H, D = 16, 64
NCORES = 8
HPC = H // NCORES          # heads per core = 2
HD = HPC * D               # hidden dims per core = 128
NT = T // 512              # 8 tq blocks of 512
NE = E // 128              # 8 e-chunks of 128
NTB = T // 128             # 32 tk blocks of 128

F32 = mybir.dt.float32
BF16 = mybir.dt.bfloat16
NPBF16 = np.dtype(ml_dtypes.bfloat16)
AF = mybir.ActivationFunctionType


def _build_kernel():
    nc = bacc.Bacc("TRN2", target_bir_lowering=False, debug=False)

    xT = nc.dram_tensor("xT", [E, T], BF16, kind="ExternalInput")
    wq = nc.dram_tensor("wq", [E, HD], BF16, kind="ExternalInput")
    wk = nc.dram_tensor("wk", [E, HD], BF16, kind="ExternalInput")
    wv = nc.dram_tensor("wv", [E, HD], BF16, kind="ExternalInput")
    bqkv = nc.dram_tensor("bqkv", [3, HD, 1], F32, kind="ExternalInput")
    wo = nc.dram_tensor("wo", [HD, E], BF16, kind="ExternalInput")
    out = nc.dram_tensor("out", [T, E], F32, kind="ExternalOutput")

    with tile.TileContext(nc) as tc:
        _body(nc, tc, xT, wq, wk, wv, bqkv, wo, out)
    nc.compile()
    return nc


def _body(nc, tc, xT, wq, wk, wv, bqkv, wo, out):
    from contextlib import ExitStack

    ctx = ExitStack()
    with ctx:
        const = ctx.enter_context(tc.tile_pool(name="const", bufs=1))
        big = ctx.enter_context(tc.tile_pool(name="big", bufs=1))
        xpool = ctx.enter_context(tc.tile_pool(name="xp", bufs=3))
        ppool = ctx.enter_context(tc.tile_pool(name="pp", bufs=4))
        opool = ctx.enter_context(tc.tile_pool(name="op", bufs=3))
        small = ctx.enter_context(tc.tile_pool(name="sm", bufs=4))
        ps_mm = ctx.enter_context(tc.tile_pool(name="ps_mm", bufs=2, space="PSUM"))
        ps_o = ctx.enter_context(tc.tile_pool(name="ps_o", bufs=3, space="PSUM"))
        ps_q = ctx.enter_context(tc.tile_pool(name="ps_q", bufs=1, space="PSUM"))

        # ---- constants / weights ----
        identb = const.tile([128, 128], BF16)
        make_identity(nc, identb[:])

        wq_sb = const.tile([128, NE, HD], BF16)
        wk_sb = const.tile([128, NE, HD], BF16)
        wv_sb = const.tile([128, NE, HD], BF16)
        for w_dram, w_sb in ((wq, wq_sb), (wk, wk_sb), (wv, wv_sb)):
            nc.sync.dma_start(
                w_sb[:], w_dram[:].rearrange("(a p) c -> p a c", p=128)
            )
        wo_sb = const.tile([128, E], BF16)
        nc.sync.dma_start(wo_sb[:], wo[:])

        bq_sb = const.tile([128, 1], F32)
        bk_sb = const.tile([128, 1], F32)
        bv_sb = const.tile([128, 1], F32)
        nc.sync.dma_start(bq_sb[:], bqkv[0])
        nc.sync.dma_start(bk_sb[:], bqkv[1])
        nc.sync.dma_start(bv_sb[:], bqkv[2])

        # q in zero-padded block-diagonal layout: qTz[0:64, 0, t] = head-0
        # q^T, qTz[64:128, 1, t] = head-1 q^T, zeros elsewhere — so S^T for
        # both heads is one K=128 matmul against kT's natural stacking.
        qTz = big.tile([128, 2, T], BF16)
        kT = big.tile([128, T], BF16)
        vT = big.tile([128, T], BF16)
        # V row-major per (tk block, head) plus a ones column.
        V2 = big.tile([128, NTB, HPC, D + 1], BF16)
        # normalized attention outputs, transposed: rows h*64+d, cols t
        UnT = big.tile([128, T], BF16)

        nc.gpsimd.memset(qTz[0:D, 1, :], 0.0)
        nc.gpsimd.memset(qTz[D:128, 0, :], 0.0)
        nc.gpsimd.memset(V2[:, :, :, D], 1.0)

        wparams = ((wq_sb, bq_sb), (wk_sb, bk_sb), (wv_sb, bv_sb))
        xs_map = {}

        def load_x(tcc):
            # one DMA for the whole [1024, 512] chunk (8 e-slices)
            ts512 = slice(tcc * 512, (tcc + 1) * 512)
            xsb = xpool.tile([128, NE, 512], BF16, tag="xsb")
            nc.sync.dma_start(
                xsb[:], xT[:, ts512].rearrange("(a p) t -> p a t", p=128)
            )
            xs_map[tcc] = xsb

        def emit_qkv(tcc, m):
            w_sb, b_sb = wparams[m]
            ts512 = slice(tcc * 512, (tcc + 1) * 512)
            ps = ps_q.tile([128, 512], F32, tag="q")
            for ec in range(NE):
                nc.tensor.matmul(
                    ps[:], w_sb[:, ec, :], xs_map[tcc][:, ec, :],
                    start=(ec == 0), stop=(ec == NE - 1),
                )
            if m == 0:
                nc.vector.tensor_scalar_add(
                    qTz[0:D, 0, ts512], ps[0:D, :], b_sb[0:D]
                )
                nc.vector.tensor_scalar_add(
                    qTz[D:128, 1, ts512], ps[D:128, :], b_sb[D:128]
                )
            else:
                dst = kT if m == 1 else vT
                nc.vector.tensor_scalar_add(dst[:, ts512], ps[:], b_sb[:])

        def emit_vtrans(tcc, j):
            # V' transpose, K=128 full-width: both heads in one go
            tb = 4 * tcc + j
            pst = ps_q.tile([128, 128], BF16, tag="q")
            nc.tensor.transpose(
                pst[:], vT[:, tb * 128:(tb + 1) * 128], identb[:]
            )
            nc.vector.tensor_copy(
                V2[:, tb, :, 0:D],
                pst[:].rearrange("p (h d) -> p h d", h=HPC),
            )

        def emit_piece(piece):
            kind = piece[0]
            if kind == "qkv":
                emit_qkv(piece[1], piece[2])
            elif kind == "vtrans":
                emit_vtrans(piece[1], piece[2])
            else:
                _outproj_tile(nc, ps_mm, opool, UnT, wo_sb, out, piece[1])

        def emit_S(qb, tb):
            psS = ps_mm.tile([128, 1024], F32, tag="mm")
            psSv = psS[:].rearrange("p (qc h f) -> p qc h f", qc=2, h=HPC)
            for qc in range(2):
                f0 = max(0, tb * 128 - qb * 512 - qc * 256)
                if f0 >= 256:
                    continue
                t0 = qb * 512 + qc * 256 + f0
                t1 = qb * 512 + (qc + 1) * 256
                nc.tensor.matmul(
                    psSv[:, qc, :, f0:256],
                    kT[:, tb * 128:(tb + 1) * 128],
                    qTz[:, :, t0:t1],
                    start=True, stop=True,
                )
            return psS, psSv

        # ---- prologue: x for chunks 0/1, full QKV chunk 0. Dummy
        # identity matmuls fill the input-DMA wait: they cost nothing the
        # PE would otherwise use, and ~4us of sustained full-array
        # activity flips the clock gate to 2.4 GHz before the real
        # matmuls start ----
        load_x(0)
        load_x(1)
        for i in range(20):
            wps = ps_q.tile([128, 128], F32, tag="q")
            nc.tensor.matmul(wps[:], identb[:], identb[:],
                             start=True, stop=True)
        for m in range(3):
            emit_qkv(0, m)
        for j in range(4):
            emit_vtrans(0, j)

        # ---- merged pipeline: per step, the attention q block `step`
        # runs its ACT-paced tk loop while the NEXT chunk's projection
        # matmuls and the lag-2 out-proj are spread through it as PE
        # filler pieces ----
        for step in range(NT):
            if step + 2 < NT:
                load_x(step + 2)
            pieces = []
            if step + 1 < NT:
                pieces += [("qkv", step + 1, m) for m in range(3)]
                pieces += [("vtrans", step + 1, j) for j in range(4)]
            if step >= 2:
                pieces += [("out", tt)
                           for tt in range((step - 2) * 4, (step - 1) * 4)]
            if step == NT - 1:
                # the second-to-last block's out-proj rides late in the
                # final tk loop (its normalize is settled well before)
                pieces += [("out", tt)
                           for tt in range((NT - 2) * 4, (NT - 1) * 4)]
            qb = step
            nblk = 4 * (qb + 1)
            emit_at = {}
            for i, piece in enumerate(pieces):
                emit_at.setdefault((i + 1) * nblk // (len(pieces) + 1),
                                   []).append(piece)
            pos = []
            for h in range(HPC):
                po = ps_o.tile([D + 1, 512], F32, tag="o")
                pos.append(po)

            def emit_O(tb, Pv):
                qc1_only = tb * 128 - qb * 512 >= 256
                for h in range(HPC):
                    if qc1_only:
                        f1 = tb * 128 - qb * 512 - 256
                        nc.tensor.matmul(
                            pos[h][:, 256 + f1:512],
                            V2[:, tb, h, :],
                            Pv[:, 1, h, f1:256],
                            start=(tb == 0), stop=(tb == nblk - 1),
                        )
                    else:
                        nc.tensor.matmul(
                            pos[h][:, :],
                            V2[:, tb, h, :],
                            Pv[:, :, h, :],
                            start=(tb == 0), stop=(tb == nblk - 1),
                        )

            Stiles = {0: emit_S(qb, 0)}
            Pprev = None
            for tb in range(nblk):
                diag = tb >= 4 * qb
                qc1_only = tb * 128 - qb * 512 >= 256
                psS, psSv = Stiles.pop(tb)
                P = ppool.tile([128, 1024], BF16, tag="P")
                Pv = P[:].rearrange("p (qc h f) -> p qc h f", qc=2, h=HPC)
                if qc1_only:
                    # only the qc1 sub-block is live: exp + mask just that
                    f1 = tb * 128 - qb * 512 - 256
                    nc.scalar.activation(
                        Pv[:, 1, :, f1:256], psSv[:, 1, :, f1:256],
                        AF.Exp, scale=0.125,
                    )
                    nc.gpsimd.affine_select(
                        out=Pv[:, 1, :, f1:256], in_=Pv[:, 1, :, f1:256],
                        compare_op=mybir.AluOpType.is_ge,
                        fill=0.0,
                        base=qb * 512 + 256 + f1 - tb * 128,
                        channel_multiplier=-1,
                        pattern=[[0, HPC], [1, 256 - f1]],
                    )
                else:
                    nc.scalar.activation(P[:], psS[:], AF.Exp, scale=0.125)
                    if diag:
                        # keep where tq >= tk:
                        # (qb*512 + qc*256 + f) - (tb*128 + p) >= 0
                        nc.gpsimd.affine_select(
                            out=Pv, in_=Pv,
                            compare_op=mybir.AluOpType.is_ge,
                            fill=0.0,
                            base=qb * 512 - tb * 128,
                            channel_multiplier=-1,
                            pattern=[[256, 2], [0, HPC], [1, 256]],
                        )
                # PE stream per iteration: S(tb+1), filler pieces, then
                # O'(tb-1) — whose exp finished an iteration ago, so the
                # PE never sits out an exp latency
                if tb + 1 < nblk:
                    Stiles[tb + 1] = emit_S(qb, tb + 1)
                for piece in emit_at.get(tb, ()):
                    emit_piece(piece)
                if Pprev is not None:
                    emit_O(tb - 1, Pprev)
                Pprev = Pv
            emit_O(nblk - 1, Pprev)
            # normalize: U = O'[0:64] * (1 / O'[64]) columnwise; the two
            # heads' chains are interleaved by stage so DVE and GpSimd
            # pipeline them instead of running them back-to-back
            drows, rbs, rbrs = [], [], []
            for h in range(HPC):
                drow = small.tile([1, 512], F32, tag="drow")
                nc.vector.tensor_copy(drow[:], pos[h][D:D + 1, :])
                drows.append(drow)
            for h in range(HPC):
                rb = small.tile([D, 512], F32, tag="rb")
                nc.gpsimd.partition_broadcast(rb[:], drows[h][:], channels=D)
                rbs.append(rb)
            for h in range(HPC):
                rbr = small.tile([D, 512], F32, tag="rbr")
                nc.vector.reciprocal_approx_fast(rbr[:], rbs[h][:])
                rbrs.append(rbr)
            for h in range(HPC):
                nc.vector.tensor_mul(
                    UnT[h * D:(h + 1) * D, qb * 512:(qb + 1) * 512],
                    pos[h][0:D, :], rbrs[h][:],
                )
        for tt in range((NT - 1) * 4, NT * 4):
            _outproj_tile(nc, ps_mm, opool, UnT, wo_sb, out, tt)


def _outproj_tile(nc, ps_mm, opool, UnT, wo_sb, out, tt):
    osb2 = opool.tile([128, E], F32, tag="out")
    for half in range(2):
        psc = ps_mm.tile([128, 1024], F32, tag="mm")
        nc.tensor.matmul(
            psc[:, 0:512],
            UnT[:, tt * 128:(tt + 1) * 128],
            wo_sb[:, half * 512:(half + 1) * 512],
            start=True, stop=True,
        )
        nc.vector.tensor_copy(
            osb2[:, half * 512:(half + 1) * 512], psc[:, 0:512]
        )
    nc.sync.dma_start(out[tt * 128:(tt + 1) * 128, :], osb2[:])


_NC_CACHE = None


def _get_nc():
    global _NC_CACHE
    if _NC_CACHE is None:
        _NC_CACHE = _build_kernel()
    return _NC_CACHE


def _make_in_maps(x, w_qkv, b_qkv, w_out):
    x2 = np.asarray(x, dtype=np.float32).reshape(T, E)
    xT = np.ascontiguousarray(x2.T).astype(NPBF16)
    w_qkv = np.asarray(w_qkv, dtype=np.float32)
    b_qkv = np.asarray(b_qkv, dtype=np.float32)
    w_out = np.asarray(w_out, dtype=np.float32)
    in_maps = []
    for c in range(NCORES):
        s = slice(c * HD, (c + 1) * HD)
        in_maps.append({
            "xT": xT,
            "wq": np.ascontiguousarray(
                w_qkv[:, 0 * E + c * HD:0 * E + (c + 1) * HD]).astype(NPBF16),
            "wk": np.ascontiguousarray(
                w_qkv[:, 1 * E + c * HD:1 * E + (c + 1) * HD]).astype(NPBF16),
            "wv": np.ascontiguousarray(
                w_qkv[:, 2 * E + c * HD:2 * E + (c + 1) * HD]).astype(NPBF16),
            "bqkv": np.ascontiguousarray(
                np.stack([
                    b_qkv[0 * E + c * HD:0 * E + (c + 1) * HD],
                    b_qkv[1 * E + c * HD:1 * E + (c + 1) * HD],
                    b_qkv[2 * E + c * HD:2 * E + (c + 1) * HD],
                ]).reshape(3, HD, 1)
            ),
            "wo": np.ascontiguousarray(w_out[s, :]).astype(NPBF16),
        })
    return in_maps


def run_sharded(x, w_qkv, b_qkv, w_out, b_out, trace=False):
    """Run the SPMD kernel; returns (full_output, BassKernelResults)."""
    nc = _get_nc()
    in_maps = _make_in_maps(x, w_qkv, b_qkv, w_out)
    res = run_bass_kernel_spmd(
        nc, in_maps, core_ids=list(range(NCORES)), trace=trace
    )
    acc = np.zeros((T, E), dtype=np.float32)
    for c in range(NCORES):
        acc += res.results[c]["out"]
    acc += np.asarray(b_out, dtype=np.float32)[None, :]
    return acc.reshape(1, T, E), res


def kernel(x, w_qkv, b_qkv, w_out, b_out):
    out, _ = run_sharded(x, w_qkv, b_qkv, w_out, b_out, trace=False)
    return out
